# revision 2
# baseline (speedup 1.0000x reference)
"""Grouped-experts SwiGLU MLP on 8 TRN2 NeuronCores, expert-parallel.

Per core (one expert):
    g = x @ gate; u = x @ down; h = silu(g) * u; out = h @ up
with T=2048, D_IN=2048, D_OUT=4096 (three 2048x2048x4096 matmuls).

Layout strategy: the host pre-transposes x[e] -> xT [D, T] so every matmul
operand arrives in its natural layout (contraction dim on partitions):
  MM1/2: psum[h,t] += gate[d,h].T-block @ xT[d,t]   (lhsT=gate tile, rhs=xT)
  MM3:   psum[t,d] += hT[h,t]-block    @ up[h,d]    (lhsT=hT tile,  rhs=up)
hT (silu(g)*u, [H, T]) is staged in DRAM between the two phases (too big
for SBUF at fp32 alongside resident xT).

All matmuls run in float32r (reduced-precision fp32, 1 PE cycle/row at
moving-dim 512 == bf16 rate; measured rel-err ~2e-4 vs fp32).
"""
import sys

if "/opt/trn_rl_repo" not in sys.path:
    sys.path.insert(0, "/opt/trn_rl_repo")

import numpy as np

import bass_rust
import concourse.bass as bass
import concourse.mybir as mybir
import concourse.tile as tile
from concourse.bass_utils import run_bass_kernel_spmd

E, T, D, H = 8, 2048, 2048, 4096
P = 128
KD = D // P   # 16 contraction chunks for MM1/2
KH = H // P   # 32 contraction chunks for MM3
NT = T // P   # 16 token stripes
F32 = mybir.dt.float32
F32R = mybir.dt.float32r
SILU = mybir.ActivationFunctionType.Silu
COPY = mybir.ActivationFunctionType.Copy


def _split_multi_waits(nc, max_waits=1):
    """This walrus build rejects instructions with >1 sync wait ("Too many
    sync wait commands").  Hoist extra waits onto single-wait NOPs on the
    same engine, placed immediately before the offending instruction."""
    ctr = 0
    for f in nc.m.functions:
        for blk in f.blocks:
            out = []
            changed = False
            for inst in blk.instructions:
                si = inst.sync_info
                waits = list(si.on_wait) if si is not None and si.on_wait else []
                if len(waits) > max_waits:
                    for w in waits[:-max_waits]:
                        ctr += 1
                        n = bass_rust.InstNoOp(name=f"I-wsplit-{ctr}")
                        n.engine = inst.engine
                        n.sync_info = bass_rust.SyncInfo(on_wait=[w], on_update=[])
                        out.append(n)
                    inst.sync_info = bass_rust.SyncInfo(
                        on_wait=waits[-max_waits:],
                        on_update=list(si.on_update or []),
                    )
                    changed = True
                out.append(inst)
            if changed:
                blk.instructions = out
    return ctr


def _build():
    nc = bass.Bass()
    xt_ext = nc.declare_dram_parameter("xt", [D, T], F32, isOutput=False)
    gate_ext = nc.declare_dram_parameter("gate", [D, H], F32, isOutput=False)
    down_ext = nc.declare_dram_parameter("down", [D, H], F32, isOutput=False)
    up_ext = nc.declare_dram_parameter("up", [H, D], F32, isOutput=False)
    out_ext = nc.declare_dram_parameter("out", [T, D], F32, isOutput=True)
    ht_dram = nc.dram_tensor("ht", [H, T], F32R)

    xt_r = xt_ext[:, :].rearrange("(k p) t -> p k t", p=P).bitcast(F32R)
    gate_r = gate_ext[:, :].rearrange("(k p) h -> p k h", p=P).bitcast(F32R)
    down_r = down_ext[:, :].rearrange("(k p) h -> p k h", p=P).bitcast(F32R)
    up_r = up_ext[:, :].rearrange("(k p) d -> p k d", p=P).bitcast(F32R)

    with tile.TileContext(nc) as tc:
        # ---- Phase 1: hT[h, t] = silu(g) * u, streamed over 32 h-stripes
        with tc.tile_pool(name="xpool", bufs=1) as xpool, \
             tc.tile_pool(name="wpool", bufs=2) as wpool, \
             tc.tile_pool(name="hpool", bufs=2) as hpool, \
             tc.tile_pool(name="spool", bufs=2) as spool, \
             tc.tile_pool(name="ps1", bufs=8, space="PSUM") as ps1:
            xbuf = xpool.tile([P, KD, T], F32R)
            nc.sync.dma_start(out=xbuf[:, :, :], in_=xt_r)

            for i in range(KH):
                gbuf = wpool.tile([P, KD, P], F32R, tag="gbuf")
                dbuf = wpool.tile([P, KD, P], F32R, tag="dbuf")
                nc.sync.dma_start(out=gbuf[:, :, :], in_=gate_r[:, :, i * P:(i + 1) * P])
                nc.sync.dma_start(out=dbuf[:, :, :], in_=down_r[:, :, i * P:(i + 1) * P])

                pg = [ps1.tile([P, 512], F32, tag="ps1", name=f"pg{t4}") for t4 in range(4)]
                pd = [ps1.tile([P, 512], F32, tag="ps1", name=f"pd{t4}") for t4 in range(4)]
                for k in range(KD):
                    st, sp = k == 0, k == KD - 1
                    for t4 in range(4):
                        nc.tensor.matmul(
                            pg[t4][:, :], lhsT=gbuf[:, k, :],
                            rhs=xbuf[:, k, t4 * 512:(t4 + 1) * 512],
                            start=st, stop=sp,
                        )
                    for t4 in range(4):
                        nc.tensor.matmul(
                            pd[t4][:, :], lhsT=dbuf[:, k, :],
                            rhs=xbuf[:, k, t4 * 512:(t4 + 1) * 512],
                            start=st, stop=sp,
                        )

                hbuf = hpool.tile([P, T], F32R)
                for t4 in range(4):
                    sg = spool.tile([P, 512], F32)
                    nc.scalar.activation(out=sg[:, :], in_=pg[t4][:, :], func=SILU, scale=1.0)
                    nc.vector.tensor_mul(
                        hbuf[:, t4 * 512:(t4 + 1) * 512], pd[t4][:, :], sg[:, :]
                    )
                nc.sync.dma_start(out=ht_dram[i * P:(i + 1) * P, :], in_=hbuf[:, :])

        # ---- Phase 2: out[t, d] = sum_h hT[h, t] * up[h, d]
        with tc.tile_pool(name="upool", bufs=1) as upool, \
             tc.tile_pool(name="cpool", bufs=3) as cpool, \
             tc.tile_pool(name="opool", bufs=4) as opool, \
             tc.tile_pool(name="ps2", bufs=8, space="PSUM") as ps2:
            for dh in range(2):
                upbuf = upool.tile([P, KH, 1024], F32R)
                nc.sync.dma_start(
                    out=upbuf[:, :, :],
                    in_=up_r[:, :, dh * 1024:(dh + 1) * 1024],
                )
                for t in range(NT):
                    htcol = cpool.tile([P, KH, P], F32R)
                    nc.sync.dma_start(
                        out=htcol[:, :, :],
                        in_=ht_dram[:, t * P:(t + 1) * P].rearrange(
                            "(k p) j -> p k j", p=P
                        ),
                    )
                    po = [ps2.tile([P, 512], F32, tag="ps2", name=f"po{d5}") for d5 in range(2)]
                    for k in range(KH):
                        st, sp = k == 0, k == KH - 1
                        for d5 in range(2):
                            nc.tensor.matmul(
                                po[d5][:, :], lhsT=htcol[:, k, :],
                                rhs=upbuf[:, k, d5 * 512:(d5 + 1) * 512],
                                start=st, stop=sp,
                            )
                    for d5 in range(2):
                        oc = opool.tile([P, 512], F32)
                        nc.scalar.activation(out=oc[:, :], in_=po[d5][:, :], func=COPY, scale=1.0)
                        nc.sync.dma_start(
                            out=out_ext[t * P:(t + 1) * P,
                                        dh * 1024 + d5 * 512:dh * 1024 + (d5 + 1) * 512],
                            in_=oc[:, :],
                        )

    _split_multi_waits(nc)
    return nc


_NC = None


def kernel(x, gate_proj, down_proj, up_proj, **run_kwargs):
    global _NC
    if _NC is None:
        _NC = _build()
    in_maps = []
    for e in range(E):
        in_maps.append({
            "xt": np.ascontiguousarray(np.asarray(x[e], dtype=np.float32).T),
            "gate": np.ascontiguousarray(np.asarray(gate_proj[e], dtype=np.float32)),
            "down": np.ascontiguousarray(np.asarray(down_proj[e], dtype=np.float32)),
            "up": np.ascontiguousarray(np.asarray(up_proj[e], dtype=np.float32)),
        })
    res = run_bass_kernel_spmd(_NC, in_maps, core_ids=list(range(E)), **run_kwargs)
    out = np.stack([res.results[e]["out"] for e in range(E)]).astype(np.float32)
    if run_kwargs:
        kernel.last_result = res
    return out


# revision 4
# speedup vs baseline: 1.0230x; 1.0230x over previous
"""Grouped-experts SwiGLU MLP on 8 TRN2 NeuronCores, expert-parallel.

Per core (one expert):
    g = x @ gate; u = x @ down; h = silu(g) * u; out = h @ up
with T=2048, D_IN=2048, D_OUT=4096 (three 2048x2048x4096 matmuls).

Layout strategy: the host pre-transposes x[e] -> xT [D, T] so every matmul
operand arrives in its natural layout (contraction dim on partitions):
  MM1/2: psum[h,t] += gate[d,h].T-block @ xT[d,t]   (lhsT=gate tile, rhs=xT)
  MM3:   psum[t,d] += hT[h,t]-block    @ up[h,d]    (lhsT=hT tile,  rhs=up)
hT (silu(g)*u, [H, T]) is staged in DRAM between the two phases (too big
for SBUF at fp32 alongside resident xT).

All matmuls run in float32r (reduced-precision fp32, 1 PE cycle/row at
moving-dim 512 == bf16 rate; measured rel-err ~2e-4 vs fp32).
"""
import sys

if "/opt/trn_rl_repo" not in sys.path:
    sys.path.insert(0, "/opt/trn_rl_repo")

import numpy as np

import bass_rust
import concourse.bass as bass
import concourse.mybir as mybir
import concourse.tile as tile
from concourse.bass_utils import run_bass_kernel_spmd

E, T, D, H = 8, 2048, 2048, 4096
P = 128
KD = D // P   # 16 contraction chunks for MM1/2
KH = H // P   # 32 contraction chunks for MM3
NT = T // P   # 16 token stripes
F32 = mybir.dt.float32
F32R = mybir.dt.float32r
SILU = mybir.ActivationFunctionType.Silu
COPY = mybir.ActivationFunctionType.Copy


def _split_multi_waits(nc, max_waits=1):
    """This walrus build rejects instructions with >1 sync wait ("Too many
    sync wait commands").  Hoist extra waits onto single-wait NOPs on the
    same engine, placed immediately before the offending instruction."""
    ctr = 0
    for f in nc.m.functions:
        for blk in f.blocks:
            out = []
            changed = False
            for inst in blk.instructions:
                si = inst.sync_info
                waits = list(si.on_wait) if si is not None and si.on_wait else []
                if len(waits) > max_waits:
                    for w in waits[:-max_waits]:
                        ctr += 1
                        n = bass_rust.InstNoOp(name=f"I-wsplit-{ctr}")
                        n.engine = inst.engine
                        n.sync_info = bass_rust.SyncInfo(on_wait=[w], on_update=[])
                        out.append(n)
                    inst.sync_info = bass_rust.SyncInfo(
                        on_wait=waits[-max_waits:],
                        on_update=list(si.on_update or []),
                    )
                    changed = True
                out.append(inst)
            if changed:
                blk.instructions = out
    return ctr


def _build():
    nc = bass.Bass()
    xt_ext = nc.declare_dram_parameter("xt", [D, T], F32, isOutput=False)
    gate_ext = nc.declare_dram_parameter("gate", [D, H], F32, isOutput=False)
    down_ext = nc.declare_dram_parameter("down", [D, H], F32, isOutput=False)
    up_ext = nc.declare_dram_parameter("up", [H, D], F32, isOutput=False)
    out_ext = nc.declare_dram_parameter("out", [T, D], F32, isOutput=True)
    ht_dram = nc.dram_tensor("ht", [H, T], F32R)

    xt_r = xt_ext[:, :].rearrange("(k p) t -> p k t", p=P).bitcast(F32R)
    gate_r = gate_ext[:, :].rearrange("(k p) h -> p k h", p=P).bitcast(F32R)
    down_r = down_ext[:, :].rearrange("(k p) h -> p k h", p=P).bitcast(F32R)
    up_r = up_ext[:, :].rearrange("(k p) d -> p k d", p=P).bitcast(F32R)

    with tile.TileContext(nc) as tc:
        # ---- Phase 1: hT[h, t] = silu(g) * u, streamed over 32 h-stripes
        with tc.tile_pool(name="xpool", bufs=1) as xpool, \
             tc.tile_pool(name="wpool", bufs=2) as wpool, \
             tc.tile_pool(name="hpool", bufs=2) as hpool, \
             tc.tile_pool(name="spool", bufs=2) as spool, \
             tc.tile_pool(name="ps1", bufs=8, space="PSUM") as ps1:
            xbuf = xpool.tile([P, KD, T], F32R)
            for k in range(KD):
                nc.sync.dma_start(out=xbuf[:, k, :], in_=xt_r[:, k, :])

            for i in range(KH):
                gbuf = wpool.tile([P, KD, P], F32R, tag="gbuf")
                dbuf = wpool.tile([P, KD, P], F32R, tag="dbuf")
                nc.sync.dma_start(out=gbuf[:, :, :], in_=gate_r[:, :, i * P:(i + 1) * P])
                nc.sync.dma_start(out=dbuf[:, :, :], in_=down_r[:, :, i * P:(i + 1) * P])

                pg = [ps1.tile([P, 512], F32, tag="ps1", name=f"pg{t4}") for t4 in range(4)]
                pd = [ps1.tile([P, 512], F32, tag="ps1", name=f"pd{t4}") for t4 in range(4)]
                for k in range(KD):
                    st, sp = k == 0, k == KD - 1
                    for t4 in range(4):
                        nc.tensor.matmul(
                            pg[t4][:, :], lhsT=gbuf[:, k, :],
                            rhs=xbuf[:, k, t4 * 512:(t4 + 1) * 512],
                            start=st, stop=sp,
                        )
                    for t4 in range(4):
                        nc.tensor.matmul(
                            pd[t4][:, :], lhsT=dbuf[:, k, :],
                            rhs=xbuf[:, k, t4 * 512:(t4 + 1) * 512],
                            start=st, stop=sp,
                        )

                hbuf = hpool.tile([P, T], F32R)
                for t4 in range(4):
                    sg = spool.tile([P, 512], F32)
                    nc.scalar.activation(out=sg[:, :], in_=pg[t4][:, :], func=SILU, scale=1.0)
                    nc.vector.tensor_mul(
                        hbuf[:, t4 * 512:(t4 + 1) * 512], pd[t4][:, :], sg[:, :]
                    )
                nc.sync.dma_start(out=ht_dram[i * P:(i + 1) * P, :], in_=hbuf[:, :])

        # ---- Phase 2: out[t, d] = sum_h hT[h, t] * up[h, d]
        with tc.tile_pool(name="upool", bufs=1) as upool, \
             tc.tile_pool(name="cpool", bufs=3) as cpool, \
             tc.tile_pool(name="opool", bufs=4) as opool, \
             tc.tile_pool(name="ps2", bufs=8, space="PSUM") as ps2:
            for dh in range(2):
                # Per-k up tiles: phase-2 matmuls start as soon as the first
                # 512 KB chunk lands instead of waiting for the whole 16 MB.
                upk = [
                    upool.tile([P, 1024], F32R, tag=f"upk{k}", name=f"up{dh}_{k}")
                    for k in range(KH)
                ]
                for k in range(KH):
                    nc.sync.dma_start(
                        out=upk[k][:, :],
                        in_=up_r[:, k, dh * 1024:(dh + 1) * 1024],
                    )
                for t in range(NT):
                    htcol = cpool.tile([P, KH, P], F32R)
                    ht_src = ht_dram[:, t * P:(t + 1) * P].rearrange(
                        "(k p) j -> p k j", p=P
                    )
                    for kc in range(4):
                        nc.sync.dma_start(
                            out=htcol[:, kc * 8:(kc + 1) * 8, :],
                            in_=ht_src[:, kc * 8:(kc + 1) * 8, :],
                        )
                    po = [ps2.tile([P, 512], F32, tag="ps2", name=f"po{d5}") for d5 in range(2)]
                    for k in range(KH):
                        st, sp = k == 0, k == KH - 1
                        for d5 in range(2):
                            nc.tensor.matmul(
                                po[d5][:, :], lhsT=htcol[:, k, :],
                                rhs=upk[k][:, d5 * 512:(d5 + 1) * 512],
                                start=st, stop=sp,
                            )
                    for d5 in range(2):
                        oc = opool.tile([P, 512], F32)
                        nc.scalar.activation(out=oc[:, :], in_=po[d5][:, :], func=COPY, scale=1.0)
                        nc.sync.dma_start(
                            out=out_ext[t * P:(t + 1) * P,
                                        dh * 1024 + d5 * 512:dh * 1024 + (d5 + 1) * 512],
                            in_=oc[:, :],
                        )

    _split_multi_waits(nc)
    return nc


_NC = None


def kernel(x, gate_proj, down_proj, up_proj, **run_kwargs):
    global _NC
    if _NC is None:
        _NC = _build()
    in_maps = []
    for e in range(E):
        in_maps.append({
            "xt": np.ascontiguousarray(np.asarray(x[e], dtype=np.float32).T),
            "gate": np.ascontiguousarray(np.asarray(gate_proj[e], dtype=np.float32)),
            "down": np.ascontiguousarray(np.asarray(down_proj[e], dtype=np.float32)),
            "up": np.ascontiguousarray(np.asarray(up_proj[e], dtype=np.float32)),
        })
    res = run_bass_kernel_spmd(_NC, in_maps, core_ids=list(range(E)), **run_kwargs)
    out = np.stack([res.results[e]["out"] for e in range(E)]).astype(np.float32)
    if run_kwargs:
        kernel.last_result = res
    return out


# revision 6
# speedup vs baseline: 1.0327x; 1.0095x over previous
"""Grouped-experts SwiGLU MLP on 8 TRN2 NeuronCores, expert-parallel.

Per core (one expert):
    g = x @ gate; u = x @ down; h = silu(g) * u; out = h @ up
with T=2048, D_IN=2048, D_OUT=4096 (three 2048x2048x4096 matmuls).

Layout strategy: the host pre-transposes x[e] -> xT [D, T] so every matmul
operand arrives in its natural layout (contraction dim on partitions):
  MM1/2: psum[h,t] += gate[d,h].T-block @ xT[d,t]   (lhsT=gate tile, rhs=xT)
  MM3:   psum[t,d] += hT[h,t]-block    @ up[h,d]    (lhsT=hT tile,  rhs=up)
hT (silu(g)*u, [H, T]) is staged in DRAM between the two phases (too big
for SBUF at fp32 alongside resident xT).

All matmuls run in float32r (reduced-precision fp32, 1 PE cycle/row at
moving-dim 512 == bf16 rate; measured rel-err ~2e-4 vs fp32).
"""
import sys

if "/opt/trn_rl_repo" not in sys.path:
    sys.path.insert(0, "/opt/trn_rl_repo")

import numpy as np

import bass_rust
import concourse.bass as bass
import concourse.mybir as mybir
import concourse.tile as tile
from concourse.bass_utils import run_bass_kernel_spmd

E, T, D, H = 8, 2048, 2048, 4096
P = 128
KD = D // P   # 16 contraction chunks for MM1/2
KH = H // P   # 32 contraction chunks for MM3
NT = T // P   # 16 token stripes
F32 = mybir.dt.float32
F32R = mybir.dt.float32r
SILU = mybir.ActivationFunctionType.Silu
COPY = mybir.ActivationFunctionType.Copy


def _split_multi_waits(nc, max_waits=1):
    """This walrus build rejects instructions with >1 sync wait ("Too many
    sync wait commands").  Hoist extra waits onto single-wait NOPs on the
    same engine, placed immediately before the offending instruction."""
    ctr = 0
    for f in nc.m.functions:
        for blk in f.blocks:
            out = []
            changed = False
            for inst in blk.instructions:
                si = inst.sync_info
                waits = list(si.on_wait) if si is not None and si.on_wait else []
                if len(waits) > max_waits:
                    for w in waits[:-max_waits]:
                        ctr += 1
                        n = bass_rust.InstNoOp(name=f"I-wsplit-{ctr}")
                        n.engine = inst.engine
                        n.sync_info = bass_rust.SyncInfo(on_wait=[w], on_update=[])
                        out.append(n)
                    inst.sync_info = bass_rust.SyncInfo(
                        on_wait=waits[-max_waits:],
                        on_update=list(si.on_update or []),
                    )
                    changed = True
                out.append(inst)
            if changed:
                blk.instructions = out
    return ctr


def _build():
    nc = bass.Bass()
    xt_ext = nc.declare_dram_parameter("xt", [D, T], F32, isOutput=False)
    gate_ext = nc.declare_dram_parameter("gate", [D, H], F32, isOutput=False)
    down_ext = nc.declare_dram_parameter("down", [D, H], F32, isOutput=False)
    up_ext = nc.declare_dram_parameter("up", [H, D], F32, isOutput=False)
    out_ext = nc.declare_dram_parameter("out", [T, D], F32, isOutput=True)
    ht_dram = nc.dram_tensor("ht", [H, T], F32R)

    xt_r = xt_ext[:, :].rearrange("(k p) t -> p k t", p=P).bitcast(F32R)
    gate_r = gate_ext[:, :].rearrange("(k p) h -> p k h", p=P).bitcast(F32R)
    down_r = down_ext[:, :].rearrange("(k p) h -> p k h", p=P).bitcast(F32R)
    up_r = up_ext[:, :].rearrange("(k p) d -> p k d", p=P).bitcast(F32R)

    with tile.TileContext(nc) as tc:
        # ---- Phase 1: hT[h, t] = silu(g) * u, streamed over 32 h-stripes
        with tc.tile_pool(name="xpool", bufs=1) as xpool, \
             tc.tile_pool(name="wpool", bufs=2) as wpool, \
             tc.tile_pool(name="hpool", bufs=2) as hpool, \
             tc.tile_pool(name="spool", bufs=2) as spool, \
             tc.tile_pool(name="ps1", bufs=8, space="PSUM") as ps1:
            # Issue stripe-0 weight loads before the bulk xT load so the first
            # matmuls aren't queued behind 16 MB of xT DMA on the same lanes.
            xbuf = xpool.tile([P, KD, T], F32R)
            gbuf0 = wpool.tile([P, KD, P], F32R, tag="gbuf", name="gbuf0")
            dbuf0 = wpool.tile([P, KD, P], F32R, tag="dbuf", name="dbuf0")
            nc.sync.dma_start(out=gbuf0[:, :, :], in_=gate_r[:, :, 0:P])
            nc.sync.dma_start(out=dbuf0[:, :, :], in_=down_r[:, :, 0:P])
            for k in range(KD):
                nc.sync.dma_start(out=xbuf[:, k, :], in_=xt_r[:, k, :])

            for i in range(KH):
                if i == 0:
                    gbuf, dbuf = gbuf0, dbuf0
                else:
                    gbuf = wpool.tile([P, KD, P], F32R, tag="gbuf", name=f"gbuf{i}")
                    dbuf = wpool.tile([P, KD, P], F32R, tag="dbuf", name=f"dbuf{i}")
                    nc.sync.dma_start(out=gbuf[:, :, :], in_=gate_r[:, :, i * P:(i + 1) * P])
                    nc.sync.dma_start(out=dbuf[:, :, :], in_=down_r[:, :, i * P:(i + 1) * P])

                pg = [ps1.tile([P, 512], F32, tag="ps1", name=f"pg{t4}") for t4 in range(4)]
                pd = [ps1.tile([P, 512], F32, tag="ps1", name=f"pd{t4}") for t4 in range(4)]
                for k in range(KD):
                    st, sp = k == 0, k == KD - 1
                    for t4 in range(4):
                        nc.tensor.matmul(
                            pg[t4][:, :], lhsT=gbuf[:, k, :],
                            rhs=xbuf[:, k, t4 * 512:(t4 + 1) * 512],
                            start=st, stop=sp,
                        )
                    for t4 in range(4):
                        nc.tensor.matmul(
                            pd[t4][:, :], lhsT=dbuf[:, k, :],
                            rhs=xbuf[:, k, t4 * 512:(t4 + 1) * 512],
                            start=st, stop=sp,
                        )

                hbuf = hpool.tile([P, T], F32R)
                for t4 in range(4):
                    sg = spool.tile([P, 512], F32)
                    nc.scalar.activation(out=sg[:, :], in_=pg[t4][:, :], func=SILU, scale=1.0)
                    nc.vector.tensor_mul(
                        hbuf[:, t4 * 512:(t4 + 1) * 512], pd[t4][:, :], sg[:, :]
                    )
                nc.sync.dma_start(out=ht_dram[i * P:(i + 1) * P, :], in_=hbuf[:, :])

        # ---- Phase 2: out[t, d] = sum_h hT[h, t] * up[h, d]
        with tc.tile_pool(name="upool", bufs=1) as upool, \
             tc.tile_pool(name="cpool", bufs=3) as cpool, \
             tc.tile_pool(name="opool", bufs=4) as opool, \
             tc.tile_pool(name="ps2", bufs=8, space="PSUM") as ps2:
            for dh in range(2):
                # Load t=0's hT column before the 16 MB of up tiles so the
                # first matmuls aren't queued behind them; per-k up tiles let
                # the k-loop start as soon as the first 512 KB chunk lands.
                htcol0 = cpool.tile([P, KH, P], F32R, tag="htcol", name=f"htcol{dh}_0")
                ht_src0 = ht_dram[:, 0:P].rearrange("(k p) j -> p k j", p=P)
                for kc in range(4):
                    nc.sync.dma_start(
                        out=htcol0[:, kc * 8:(kc + 1) * 8, :],
                        in_=ht_src0[:, kc * 8:(kc + 1) * 8, :],
                    )
                upk = [
                    upool.tile([P, 1024], F32R, tag=f"upk{k}", name=f"up{dh}_{k}")
                    for k in range(KH)
                ]
                for k in range(KH):
                    nc.sync.dma_start(
                        out=upk[k][:, :],
                        in_=up_r[:, k, dh * 1024:(dh + 1) * 1024],
                    )
                for t in range(NT):
                    if t == 0:
                        htcol = htcol0
                    else:
                        htcol = cpool.tile([P, KH, P], F32R, tag="htcol", name=f"htcol{dh}_{t}")
                        ht_src = ht_dram[:, t * P:(t + 1) * P].rearrange(
                            "(k p) j -> p k j", p=P
                        )
                        for kc in range(4):
                            nc.sync.dma_start(
                                out=htcol[:, kc * 8:(kc + 1) * 8, :],
                                in_=ht_src[:, kc * 8:(kc + 1) * 8, :],
                            )
                    po = [ps2.tile([P, 512], F32, tag="ps2", name=f"po{d5}") for d5 in range(2)]
                    for k in range(KH):
                        st, sp = k == 0, k == KH - 1
                        for d5 in range(2):
                            nc.tensor.matmul(
                                po[d5][:, :], lhsT=htcol[:, k, :],
                                rhs=upk[k][:, d5 * 512:(d5 + 1) * 512],
                                start=st, stop=sp,
                            )
                    for d5 in range(2):
                        oc = opool.tile([P, 512], F32)
                        nc.scalar.activation(out=oc[:, :], in_=po[d5][:, :], func=COPY, scale=1.0)
                        nc.sync.dma_start(
                            out=out_ext[t * P:(t + 1) * P,
                                        dh * 1024 + d5 * 512:dh * 1024 + (d5 + 1) * 512],
                            in_=oc[:, :],
                        )

    _split_multi_waits(nc)
    return nc


_NC = None


def kernel(x, gate_proj, down_proj, up_proj, **run_kwargs):
    global _NC
    if _NC is None:
        _NC = _build()
    in_maps = []
    for e in range(E):
        in_maps.append({
            "xt": np.ascontiguousarray(np.asarray(x[e], dtype=np.float32).T),
            "gate": np.ascontiguousarray(np.asarray(gate_proj[e], dtype=np.float32)),
            "down": np.ascontiguousarray(np.asarray(down_proj[e], dtype=np.float32)),
            "up": np.ascontiguousarray(np.asarray(up_proj[e], dtype=np.float32)),
        })
    res = run_bass_kernel_spmd(_NC, in_maps, core_ids=list(range(E)), **run_kwargs)
    out = np.stack([res.results[e]["out"] for e in range(E)]).astype(np.float32)
    if run_kwargs:
        kernel.last_result = res
    return out
